# revision 14
# baseline (speedup 1.0000x reference)
"""Trainium2 Bass kernel for grouped (4 kv-group) causal self-attention with
a 1024-wide sliding window, RoPE, fused QKV projection and output projection.

Problem shapes (hardcoded): B=2, T=2048, C=2048, H=16, G=4, HS=128, SWS=1024.

Sharding over the 8 NeuronCores: core = b*4 + g — data-parallel over the
batch (2) and tensor-parallel over the 4 kv groups. Each core computes its
group's QKV projection (768 cols), RoPE, the 4 query heads' sliding-window
attention, and a partial output projection against its group's 512 columns
of W_proj; the host sums the 4 group partials per batch element.

Per-core kernel design (all PE matmuls in float32r — full rate at free >= 256;
operand tensors are declared float32r so every producer rounds on write,
which the BIR verifier requires):
  - one fully interleaved loop over 256-token chunks: qkv projection ->
    RoPE -> v transpose -> 4 heads' attention -> output projection, with
    k/v kept in a rolling 5-chunk ring (sliding window = 4 chunks back)
  - everything lives transposed: x^T [C,T], qkv^T [cols,T], cos/sin^T [HS,T]
  - RoPE rotate-half = PE matmul against a +-1 permutation matrix, then two
    multiplies and an add on the vector engine (in place on qkv^T)
  - scores computed transposed (S^T[j,i] = k_j . q_i) so that:
      * exp runs on the scalar engine straight out of PSUM into SBUF
      * the softmax denominator is an all-ones matmul on the PE (broadcast
        across partitions for free), reciprocal + multiply on vector engine
      * P^T feeds the y^T matmul directly (v natural-layout as stationary)
  - sliding-window/causal masking: gpsimd affine_select zeroing the post-exp
    P^T tiles (only the tiles crossing the diagonal or the window edge)
  - output projection consumes y^T directly as the stationary operand.
"""

import numpy as np
from contextlib import ExitStack

import concourse.bass as bass
import concourse.mybir as mybir
import concourse.tile as tile
from concourse import bacc
from concourse.bass_utils import run_bass_kernel_spmd
from concourse.masks import make_identity

F32 = mybir.dt.float32
F32R = mybir.dt.float32r
AF = mybir.ActivationFunctionType
ALU = mybir.AluOpType

B, T, C, HS, NQ, G = 2, 2048, 2048, 128, 4, 4
G_COLS = 768  # per group: 4*128 q cols + 128 k + 128 v
SWS = 1024
SCALE = 1.0 / float(np.sqrt(np.float32(HS)))


def build_attention_nc(CHUNK=256, CC=512, DT=F32R, reps=1, xbufs=2, pbufs=6,
                       ybufs=2, rbufs=3, obufs=4, rtbufs=4,
                       fF=5, fY=1, fD=1, fP=1):
    CT = C // 128          # 16 contraction tiles for the qkv projection
    NCH = T // CHUNK       # chunks
    JPC = CHUNK // 128     # 128-wide j-tiles per chunk
    RING = SWS // CHUNK + 1  # k/v chunks alive (window + current)

    nc = bacc.Bacc("TRN2", target_bir_lowering=False, debug=False)
    xT = nc.dram_tensor("xT", [C, T], DT, kind="ExternalInput").ap()
    wqkvT = nc.dram_tensor("wqkvT", [C, G_COLS], DT, kind="ExternalInput").ap()
    cosT = nc.dram_tensor("cosT", [HS, T], F32, kind="ExternalInput").ap()
    sinT = nc.dram_tensor("sinT", [HS, T], F32, kind="ExternalInput").ap()
    wprojT = nc.dram_tensor("wprojT", [NQ * HS, C], DT, kind="ExternalInput").ap()
    outp = nc.dram_tensor("outp", [T, C], F32, kind="ExternalOutput").ap()

    xT_r = xT.rearrange("(co p) t -> p co t", p=128)      # [128, 16, T]
    wq_r = wqkvT.rearrange("(co p) n -> p co n", p=128)   # [128, 16, 768]
    wp_r = wprojT.rearrange("(h p) c -> p h c", p=128)    # [128, 4, C]
    out_r = outp.rearrange("(to p) c -> p to c", p=128)   # [128, 16, C]

    with tile.TileContext(nc) as tc, ExitStack() as ctx:
        const = ctx.enter_context(tc.tile_pool(name="const", bufs=1))
        wpool = ctx.enter_context(tc.tile_pool(name="wpool", bufs=1))
        qkvp = ctx.enter_context(tc.tile_pool(name="qkvp", bufs=RING))
        vpool = ctx.enter_context(tc.tile_pool(name="vpool", bufs=RING))
        xpool = ctx.enter_context(tc.tile_pool(name="xpool", bufs=xbufs))
        cspool = ctx.enter_context(tc.tile_pool(name="cspool", bufs=2))
        rtmp = ctx.enter_context(tc.tile_pool(name="rtmp", bufs=rtbufs))
        ppool = ctx.enter_context(tc.tile_pool(name="ppool", bufs=pbufs))
        ypool = ctx.enter_context(tc.tile_pool(name="ypool", bufs=ybufs))
        rpool = ctx.enter_context(tc.tile_pool(name="rpool", bufs=rbufs))
        opool = ctx.enter_context(tc.tile_pool(name="opool", bufs=obufs))
        # PSUM: 8 banks total -> flow 3 + y 2 + den 1 + proj 2
        ps_flow = ctx.enter_context(tc.tile_pool(name="psF", bufs=fF, space="PSUM"))
        ps_y = ctx.enter_context(tc.tile_pool(name="psY", bufs=fY, space="PSUM"))
        ps_d = ctx.enter_context(tc.tile_pool(name="psD", bufs=fD, space="PSUM"))
        ps_p = ctx.enter_context(tc.tile_pool(name="psP", bufs=fP, space="PSUM"))

        # rotate-half permutation, transposed: protT[p, f] = Prot[f, p].
        # gpsimd builds the f32 version; a DVE copy rounds into DT (verifier
        # requires a rounding producer for fp32r matmul operands).
        protT_f = const.tile([128, 128], F32, tag="protT_f")
        nc.gpsimd.memset(protT_f[:], 0.0)
        nc.gpsimd.affine_select(protT_f[:], protT_f[:], pattern=[[-1, 128]],
                                compare_op=ALU.not_equal, fill=-1.0,
                                base=-64, channel_multiplier=1)
        nc.gpsimd.affine_select(protT_f[:], protT_f[:], pattern=[[-1, 128]],
                                compare_op=ALU.not_equal, fill=1.0,
                                base=64, channel_multiplier=1)
        protT = const.tile([128, 128], DT, tag="protT")
        nc.vector.tensor_copy(out=protT[:], in_=protT_f[:])

        ident_f = const.tile([128, 128], F32, tag="ident_f")
        make_identity(nc, ident_f[:])
        ident = const.tile([128, 128], DT, tag="ident")
        nc.vector.tensor_copy(out=ident[:], in_=ident_f[:])

        onesf_f = const.tile([128, 128], F32, tag="onesf_f")
        nc.vector.memset(onesf_f[:], 1.0)
        onesf = const.tile([128, 128], DT, tag="onesf")
        nc.vector.tensor_copy(out=onesf[:], in_=onesf_f[:])

        for _rep in range(reps):
            # weights: qkv weight split into 8 DMA parts so the first
            # projection matmuls start as soon as part 0 lands; the proj
            # weight is queued later (first needed ~chunk 0's projection)
            w_sb = wpool.tile([128, CT, G_COLS], DT, tag="bigw")
            for wp8 in range(8):
                nc.sync.dma_start(w_sb[:, wp8 * 2:(wp8 + 1) * 2, :],
                                  wq_r[:, wp8 * 2:(wp8 + 1) * 2, :])
            wp_sb = wpool.tile([128, NQ, C], DT, tag="bigwp")

            ring_qkv = [None] * NCH
            ring_v = [None] * NCH

            for icx in range(NCH):
                i0 = icx * CHUNK
                tsl = slice(i0, i0 + CHUNK)
                # --- qkv projection for this chunk ---
                xt = xpool.tile([128, CT, CHUNK], DT, tag="xT")
                nc.sync.dma_start(xt[:, 0:8, :], xT_r[:, 0:8, tsl])
                nc.sync.dma_start(xt[:, 8:16, :], xT_r[:, 8:16, tsl])
                cost = cspool.tile([128, CHUNK], F32, tag="cosT")
                nc.sync.dma_start(cost[:], cosT[:, tsl])
                sint = cspool.tile([128, CHUNK], F32, tag="sinT")
                nc.sync.dma_start(sint[:], sinT[:, tsl])

                qkv_c = qkvp.tile([128, 6, CHUNK], DT, tag="qkvT")
                v_c = vpool.tile([128, JPC, HS], DT, tag="vnat")
                ring_qkv[icx] = qkv_c
                ring_v[icx] = v_c
                if icx == 0:
                    # queue proj weight behind chunk 0's inputs (4 parts) —
                    # first consumed by chunk 0's output projection
                    for wp4 in range(4):
                        nc.sync.dma_start(wp_sb[:, wp4, :], wp_r[:, wp4, :])

                for m in range(6):
                    ps = ps_flow.tile([128, CHUNK], F32, tag="flow")
                    for ck in range(CT):
                        nc.tensor.matmul(ps[:], w_sb[:, ck, m * 128:(m + 1) * 128],
                                         xt[:, ck, :],
                                         start=(ck == 0), stop=(ck == CT - 1))
                    nc.scalar.copy(out=qkv_c[:, m, :], in_=ps[:])
                # --- rope (in place) on q heads + k ---
                for h in range(5):
                    qsl = qkv_c[:, h, :]
                    psr = ps_flow.tile([128, CHUNK], F32, tag="flow")
                    nc.tensor.matmul(psr[:], protT[:], qsl, start=True, stop=True)
                    tmp = rtmp.tile([128, CHUNK], F32, tag="ropetmp")
                    nc.gpsimd.tensor_mul(tmp[:], qsl, cost[:])
                    nc.vector.tensor_mul(qsl, psr[:], sint[:])
                    nc.vector.tensor_add(qsl, qsl, tmp[:])
                # --- v back to natural [t, d] layout ---
                for jt in range(JPC):
                    pst = ps_flow.tile([128, 128], DT, tag="flow")
                    nc.tensor.transpose(pst[:], qkv_c[:, 5, jt * 128:(jt + 1) * 128],
                                        ident[:])
                    nc.scalar.copy(out=v_c[:, jt, :], in_=pst[:])

                # --- attention for the 4 heads of this chunk ---
                jt_lo = max(0, i0 - (SWS - 1)) // 128
                jt_hi = (i0 + CHUNK - 1) // 128
                yt = ypool.tile([128, NQ, CHUNK], DT, tag="yTc")
                for h in range(NQ):
                    psy = ps_y.tile([128, CHUNK], F32, tag="y")
                    psd = ps_d.tile([128, CHUNK], F32, tag="d")
                    # j-tiles are processed in pairs: both score matmuls land
                    # in one PSUM bank (sequential single-matmul groups), and
                    # one exp covers both halves — halves ACT's fixed costs.
                    for jp in range(jt_lo, jt_hi + 1, 2):
                        psS = ps_flow.tile([128, 2 * CHUNK], F32, tag="flow",
                                           name="psS")
                        pt = ppool.tile([128, 2 * CHUNK], DT, tag="PT", name="pt")
                        for js2 in range(2):
                            jt = jp + js2
                            jc, js = jt // JPC, jt % JPC
                            kT_t = ring_qkv[jc][:, 4, js * 128:(js + 1) * 128]
                            nc.tensor.matmul(
                                psS[:, js2 * CHUNK:(js2 + 1) * CHUNK],
                                kT_t, qkv_c[:, h, :], start=True, stop=True)
                        nc.scalar.activation(pt[:], psS[:], AF.Exp, scale=SCALE)
                        for js2 in range(2):
                            jt = jp + js2
                            jc, js = jt // JPC, jt % JPC
                            v_t = ring_v[jc][:, js, :]
                            pth = pt[:, js2 * CHUNK:(js2 + 1) * CHUNK]
                            off = jt * 128 - i0
                            if off >= 0:
                                # causal: keep iff f - p - off >= 0  (i >= j)
                                nc.gpsimd.affine_select(
                                    pth, pth, pattern=[[1, CHUNK]],
                                    compare_op=ALU.is_ge, fill=0.0,
                                    base=-off, channel_multiplier=-1)
                            base_e = off + SWS
                            if base_e < CHUNK:
                                # window edge: keep iff p - f + base_e > 0
                                nc.gpsimd.affine_select(
                                    pth, pth, pattern=[[-1, CHUNK]],
                                    compare_op=ALU.is_gt, fill=0.0,
                                    base=base_e, channel_multiplier=1)
                            first = jt == jt_lo
                            last = jt == jt_hi
                            nc.tensor.matmul(psy[:], v_t, pth,
                                             start=first, stop=last)
                            nc.tensor.matmul(psd[:], onesf[:], pth,
                                             start=first, stop=last)
                    rec = rpool.tile([128, CHUNK], F32, tag="recip")
                    nc.vector.reciprocal(rec[:], psd[:])
                    nc.vector.tensor_mul(yt[:, h, :], psy[:], rec[:])
                # --- output projection for this chunk's rows ---
                for tt in range(JPC):
                    tg = icx * JPC + tt
                    for ccx in range(C // CC):
                        psp = ps_p.tile([128, CC], F32, tag="proj")
                        for h in range(NQ):
                            nc.tensor.matmul(psp[:],
                                             yt[:, h, tt * 128:(tt + 1) * 128],
                                             wp_sb[:, h, ccx * CC:(ccx + 1) * CC],
                                             start=(h == 0), stop=(h == NQ - 1))
                        ost = opool.tile([128, CC], F32, tag="ostg")
                        nc.vector.tensor_copy(out=ost[:], in_=psp[:])
                        nc.sync.dma_start(out_r[:, tg, ccx * CC:(ccx + 1) * CC], ost[:])

    nc.compile()
    return nc


def shard_inputs(x, cos, sin, W_attn, W_proj):
    """Full inputs -> list of 8 per-core input dicts (core = b*4 + g)."""
    in_maps = []
    cosT = np.ascontiguousarray(np.asarray(cos, dtype=np.float32).T)
    sinT = np.ascontiguousarray(np.asarray(sin, dtype=np.float32).T)
    x = np.asarray(x, dtype=np.float32)
    W_attn = np.asarray(W_attn, dtype=np.float32)
    W_proj = np.asarray(W_proj, dtype=np.float32)
    for b in range(B):
        xTb = np.ascontiguousarray(x[b].T)
        for g in range(G):
            in_maps.append({
                "xT": xTb,
                "wqkvT": np.ascontiguousarray(W_attn[g * G_COLS:(g + 1) * G_COLS].T),
                "cosT": cosT,
                "sinT": sinT,
                "wprojT": np.ascontiguousarray(
                    W_proj[:, g * NQ * HS:(g + 1) * NQ * HS].T),
            })
    return in_maps


def unshard_output(results):
    out = np.zeros((B, T, C), np.float32)
    for b in range(B):
        for g in range(G):
            out[b] += results[b * G + g]["outp"]
    return out


_NC_CACHE = {}


def get_nc():
    if "nc" not in _NC_CACHE:
        _NC_CACHE["nc"] = build_attention_nc()
    return _NC_CACHE["nc"]


def kernel(x, cos, sin, W_attn, W_proj):
    in_maps = shard_inputs(x, cos, sin, W_attn, W_proj)
    nc = get_nc()
    res = run_bass_kernel_spmd(nc, in_maps, core_ids=list(range(8)))
    return unshard_output(res.results)
